# revision 36
# baseline (speedup 1.0000x reference)
"""Distributed Trainium2 (Bass/Tile) kernel for the DQN-style GNN message-passing
module.

Full-input contract: ``kernel(**inputs)`` takes the unsharded inputs exactly as
produced by ``setup_inputs()`` and returns the full output (shape ``(1,)``).

Strategy (v3 — replicated iterations, single tiny AllGather):
  - graph [N, N] row-sharded across 8 cores -> [R, N] per core (R = N/8).
  - Each core streams its shard once and computes per-row chunked sums
    s_abs_k[r] (vector tensor_reduce w/ absolute value) and s_sum_k[r]
    (scalar Copy-activation accumulate), transposes them per 128-row block
    (tensor engine), then combines the chunks with a tiny ones-matmul into
    [s_abs; s_sum] = [2, R] and casts to bf16.
  - ONE AllGather of 4KB/rank shares all row sums; every core then runs ALL
    T iterations + the readout REPLICATED with zero further collectives (the
    v1 baseline instead serialized 4 AllReduces at 10-40us each after
    streaming).

Math (the reference's exact relu identity):
  c[r, :] = s_abs[r] * A + s_sum[r] * B,  A = 0.5*|theta4| @ theta3,
                                          B = 0.5* theta4  @ theta3
  a[r, :] = xf[r] * theta1

  One state tile US [98, N] (bf16) stacks u^T (rows 0:64) over xf (row 64),
  zero padding (rows 65:96 — engine APs need 32-aligned partition starts),
  and [s_abs; s_sum] (rows 96:98).  With the host-built stationary
  M = [-theta2; theta1; 0...; A; B], one matmul computes
    pre^T = M^T @ US = (a + c - u @ theta2)^T
  and u' = relu(pre + z), z = (S_t @ theta2)^T, where S_t = sum_n u_t[n] is
  accumulated locally by the relu pass.  Matmul time is free-dim-bound, so
  the extra contraction rows are free.  u is updated in place (the Tile
  framework tracks subtile WAR/RAW deps per 512-col chunk).
"""

from contextlib import ExitStack

import ml_dtypes
import numpy as np

import concourse.bass as bass
import concourse.tile as tile
from concourse import bacc, mybir
from concourse.bass_utils import run_bass_kernel_spmd

F32 = mybir.dt.float32
BF16 = mybir.dt.bfloat16
AX = mybir.AxisListType
ALU = mybir.AluOpType
ACTF = mybir.ActivationFunctionType
BF16_NP = ml_dtypes.bfloat16

N_CORES = 8
DIM = 64
NCH = 8  # column chunks per row during streaming (chunk = C // NCH columns)
DEBUG_DUMP = False  # extra outputs for numerical debugging

_program_cache: dict = {}


def build_program(R: int, C: int, D: int, T: int, v: int, n_cores: int = N_CORES):
    """Build + compile the per-core SPMD Bass program.

    R: local rows (N / n_cores), C: row length (= N), D: dim, T: iterations,
    v: readout node index (baked in; it is a host-side scalar input).
    """
    assert R % 128 == 0 and D == 64 and C % NCH == 0
    NB = R // 128  # 128-row blocks per core
    CH = C // NCH  # columns per chunk
    # State-tile rows: [u (0:64); xf (64); zero pad (65:96); s_abs; s_sum].
    SOFF = 96
    KM = SOFF + 2  # 98
    FB = 512  # matmul chunk (one PSUM bank of f32)
    RB = 1024  # relu block (2 matmul chunks per activation instruction)
    NBLK = C // RB

    nc = bacc.Bacc(
        "TRN2",
        target_bir_lowering=False,
        debug=False,
        enable_asserts=True,
        num_devices=n_cores,
    )
    g_d = nc.dram_tensor("g", [R, C], F32, kind="ExternalInput")
    xf_d = nc.dram_tensor("xf", [1, C], BF16, kind="ExternalInput")
    mneg_d = nc.dram_tensor("mneg", [KM, D], BF16, kind="ExternalInput")
    t2_d = nc.dram_tensor("t2", [D, D], F32, kind="ExternalInput")
    t6_d = nc.dram_tensor("t6", [D, D], F32, kind="ExternalInput")
    t7_d = nc.dram_tensor("t7", [D, D], BF16, kind="ExternalInput")
    t5c_d = nc.dram_tensor("t5c", [2 * D, 1], F32, kind="ExternalInput")
    out_d = nc.dram_tensor("out", [1, 1], F32, kind="ExternalOutput")
    if DEBUG_DUMP:
        sag_d = nc.dram_tensor("sag_dump", [2 * NCH, R], BF16, kind="ExternalOutput")
        sdump_d = nc.dram_tensor("s_dump", [D, T], F32, kind="ExternalOutput")
    ident_d = nc.inline_tensor(np.eye(128, dtype=np.float32), name="ident")
    cmb_np = np.zeros((2 * NCH, 2), np.float32)
    cmb_np[:NCH, 0] = 1.0
    cmb_np[NCH:, 1] = 1.0
    cmb_d = nc.inline_tensor(cmb_np.astype(BF16_NP), name="cmb")

    rg = [list(range(n_cores))]

    with ExitStack() as ctx:
        tc = ctx.enter_context(tile.TileContext(nc))
        const = ctx.enter_context(tc.tile_pool(name="const", bufs=1))
        small = ctx.enter_context(tc.tile_pool(name="small", bufs=1))
        gp = ctx.enter_context(tc.tile_pool(name="gp", bufs=4))
        stp = ctx.enter_context(tc.tile_pool(name="stp", bufs=2))
        spp = ctx.enter_context(tc.tile_pool(name="spp", bufs=2))
        slp = ctx.enter_context(tc.tile_pool(name="sl", bufs=2))
        dram = ctx.enter_context(tc.tile_pool(name="dram", bufs=1, space="DRAM"))

        # ---- warm-up collectives (absorb ncfw cold-start under streaming;
        # the mesh collective reaches steady-state latency after ~2 ops)
        dwi = dram.tile([2, 128], BF16, tag="dwi")
        dwo = dram.tile([2 * n_cores, 128], BF16, tag="dwo")
        for _ in range(2):
            nc.gpsimd.collective_compute(
                "AllGather",
                ALU.bypass,
                replica_groups=rg,
                ins=[dwi[:].opt()],
                outs=[dwo[:].opt()],
            )

        # ---- constants / persistent tiles
        ident = const.tile([128, 128], F32)
        nc.scalar.dma_start(out=ident[:], in_=ident_d.ap())
        cmb = const.tile([2 * NCH, 2], BF16)
        nc.scalar.dma_start(out=cmb[:], in_=cmb_d.ap())
        Mb = const.tile([KM, D], BF16)
        nc.scalar.dma_start(out=Mb[:], in_=mneg_d.ap())
        t2 = const.tile([D, D], F32)
        nc.scalar.dma_start(out=t2[:], in_=t2_d.ap())
        t6 = const.tile([D, D], F32)
        nc.scalar.dma_start(out=t6[:], in_=t6_d.ap())
        t7b = const.tile([D, D], BF16)
        nc.scalar.dma_start(out=t7b[:], in_=t7_d.ap())
        t5c = const.tile([2 * D, 1], F32)
        nc.scalar.dma_start(out=t5c[:], in_=t5c_d.ap())
        z0 = const.tile([D, 1], F32)
        nc.vector.memset(z0[:], 0.0)
        zfb = const.tile([D, RB], F32)
        nc.vector.memset(zfb[:], 0.0)

        # single in-place state tile
        US = small.tile([KM, C], BF16)
        nc.vector.memset(US[0:D, :], 0.0)
        nc.vector.memset(US[D:SOFF, :], 0.0)
        nc.scalar.dma_start(out=US[D : D + 1, :], in_=xf_d.ap())

        # per-core chunked row sums, transposed: rows = [abs_0..; sum_0..]
        SAg = small.tile([2 * NCH, R], BF16)
        cinS = small.tile([2, R], BF16)

        # ---- phase 1: stream graph, reduce rows, transpose per block
        with tc.tile_pool(name="psT", bufs=2, space="PSUM") as psT, tc.tile_pool(
            name="psC", bufs=2, space="PSUM"
        ) as psC:
            for b in range(NB):
                gt = gp.tile([128, C], F32, tag="gt")
                nsub = 4 if b == NB - 1 else 2
                sw = C // nsub
                for h in range(nsub):
                    nc.sync.dma_start(
                        out=gt[:, h * sw : (h + 1) * sw],
                        in_=g_d.ap()[b * 128 : (b + 1) * 128, h * sw : (h + 1) * sw],
                    )
                SPA = spp.tile([128, 2 * NCH], F32, tag="spa")
                for k in range(NCH):
                    nc.vector.tensor_reduce(
                        out=SPA[:, k : k + 1],
                        in_=gt[:, k * CH : (k + 1) * CH],
                        axis=AX.X,
                        op=ALU.add,
                        apply_absolute_value=True,
                    )
                    st = stp.tile([128, CH], BF16, tag="st")
                    nc.scalar.activation(
                        out=st[:],
                        in_=gt[:, k * CH : (k + 1) * CH],
                        func=ACTF.Copy,
                        accum_out=SPA[:, NCH + k : NCH + k + 1],
                    )
                tb = psT.tile([2 * NCH, 128], F32, tag="tb")
                nc.tensor.transpose(out=tb[:], in_=SPA[:], identity=ident[:])
                nc.vector.tensor_copy(SAg[:, b * 128 : (b + 1) * 128], tb[:])
                if b == NB // 2 - 1 or b == NB - 1:
                    # combine chunk sums -> [s_abs; s_sum] bf16 as soon as a
                    # half of SAg is ready (avoids tensor-queue HOL blocking)
                    h = 0 if b == NB // 2 - 1 else FB
                    cp = psC.tile([2, FB], F32, tag="cp")
                    nc.tensor.matmul(
                        cp[:], lhsT=cmb[:], rhs=SAg[:, h : h + FB],
                        start=True, stop=True,
                    )
                    nc.vector.tensor_copy(cinS[:, h : h + FB], cp[:])
            if DEBUG_DUMP:
                nc.scalar.dma_start(out=sag_d.ap(), in_=SAg[:])

        # ---- share row sums: one tiny AllGather
        cin = dram.tile([2, R], BF16, tag="cin")
        cout = dram.tile([2 * n_cores, R], BF16, tag="cout")
        nc.sync.dma_start(out=cin[:], in_=cinS[:])
        nc.gpsimd.collective_compute(
            "AllGather",
            ALU.bypass,
            replica_groups=rg,
            ins=[cin[:].opt()],
            outs=[cout[:].opt()],
        )
        for i in range(n_cores):
            nc.sync.dma_start(
                out=US[SOFF:KM, i * R : (i + 1) * R],
                in_=cout[2 * i : 2 * i + 2, :],
            )
        # keep the PE HAM-warm through the AllGather window so the iteration
        # matmuls run at 2.4 GHz (reads cinS, so this schedules after phase 1)
        with tc.tile_pool(name="psW", bufs=1, space="PSUM") as psW:
            jp = psW.tile([D, FB], F32, tag="jp")
            for _ in range(20):
                nc.tensor.matmul(
                    jp[:], lhsT=cinS[0:2, 0:D], rhs=cinS[:, 0:FB],
                    start=True, stop=True,
                )

        # ---- T replicated iterations over all C nodes (u updated in place)
        psI_ctx = tc.tile_pool(name="psI", bufs=3, space="PSUM")
        psZ_ctx = tc.tile_pool(name="psZ", bufs=1, space="PSUM")
        psI = psI_ctx.__enter__()
        psZ = psZ_ctx.__enter__()
        zs = z0
        S_last = None
        if DEBUG_DUMP:
            sdt = small.tile([D, T], F32)
        for t in range(T):
            SL = slp.tile([D, NBLK], F32, tag="SL", name=f"SL{t}")
            for blk in range(NBLK):
                cols = slice(blk * RB, (blk + 1) * RB)
                ps = psI.tile([D, RB], F32, tag="ps")
                for h in range(2):
                    nc.tensor.matmul(
                        ps[:, h * FB : (h + 1) * FB],
                        lhsT=Mb[:],
                        rhs=US[:, blk * RB + h * FB : blk * RB + (h + 1) * FB],
                        start=True,
                        stop=True,
                    )
                if blk % 2 == 0:
                    nc.scalar.activation(
                        out=US[0:D, cols],
                        in_=ps[:],
                        func=ACTF.Relu,
                        bias=zs[:, 0:1],
                        accum_out=SL[:, blk : blk + 1],
                    )
                else:
                    # relu(ps + z) on DVE: (ps add z) max zeros, accum -> SL
                    nc.vector.scalar_tensor_tensor(
                        out=US[0:D, cols],
                        in0=ps[:],
                        scalar=zs[:, 0:1],
                        in1=zfb[:],
                        op0=ALU.add,
                        op1=ALU.max,
                        accum_out=SL[:, blk : blk + 1],
                    )
            S = slp.tile([D, 1], F32, tag="S", name=f"S{t}")
            nc.vector.tensor_reduce(out=S[:], in_=SL[:], axis=AX.X, op=ALU.add)
            S_last = S
            if DEBUG_DUMP:
                nc.vector.tensor_copy(sdt[:, t : t + 1], S[:])
            if t < T - 1:
                zp = psZ.tile([D, 1], F32, tag="zp")
                nc.tensor.matmul(zp[:], lhsT=t2[:], rhs=S[:], start=True, stop=True)
                znew = slp.tile([D, 1], F32, tag="zs", name=f"zs{t}")
                nc.scalar.copy(znew[:], zp[:])
                zs = znew
        psZ_ctx.__exit__(None, None, None)
        psI_ctx.__exit__(None, None, None)
        if DEBUG_DUMP:
            nc.scalar.dma_start(out=sdump_d.ap(), in_=sdt[:])

        # ---- final readout (fully local; u4 = US rows 0:64, S4 = S_last)
        with tc.tile_pool(name="psF", bufs=1, space="PSUM") as psF:
            q = psF.tile([2 * D, 1], F32, tag="q")
            nc.tensor.matmul(
                q[0:D, :], lhsT=t6[:], rhs=S_last[:], start=True, stop=True
            )
            nc.tensor.matmul(
                q[D : 2 * D, :],
                lhsT=t7b[:],
                rhs=US[0:D, v : v + 1],
                start=True,
                stop=True,
            )
            rq = small.tile([2 * D, 1], F32)
            nc.scalar.activation(out=rq[:], in_=q[:], func=ACTF.Relu)
            res = psF.tile([1, 1], F32, tag="res")
            nc.tensor.matmul(res[:], lhsT=rq[:], rhs=t5c[:], start=True, stop=True)
            ress = small.tile([1, 1], F32)
            nc.scalar.copy(ress[:], res[:])
            nc.scalar.dma_start(out=out_d.ap(), in_=ress[:])

    nc.compile()
    return nc


def get_program(R: int, C: int, D: int, T: int, v: int, n_cores: int = N_CORES):
    key = (R, C, D, T, v, n_cores)
    if key not in _program_cache:
        _program_cache[key] = build_program(R, C, D, T, v, n_cores)
    return _program_cache[key]


def make_in_maps(graph, x, theta1, theta2, theta3, theta4, theta5, theta6, theta7,
                 n_cores: int = N_CORES):
    """Host-side sharding + tiny theta preprocessing."""
    N = graph.shape[0]
    D = theta1.shape[1]
    R = N // n_cores
    f32 = np.float32

    t4 = np.asarray(theta4, f32)[0]
    t3 = np.asarray(theta3, f32)
    A = 0.5 * (np.abs(t4) @ t3)
    B = 0.5 * (t4 @ t3)
    t2 = np.ascontiguousarray(np.asarray(theta2, f32))
    mneg = np.ascontiguousarray(
        np.concatenate(
            [-t2, np.asarray(theta1, f32), np.zeros((31, D), f32),
             A[None, :], B[None, :]],
            axis=0,
        )
    ).astype(BF16_NP)  # (98, D)
    t5c = np.ascontiguousarray(np.asarray(theta5, f32).reshape(2 * D, 1))
    t6 = np.ascontiguousarray(np.asarray(theta6, f32))
    t7 = np.ascontiguousarray(np.asarray(theta7, f32)).astype(BF16_NP)
    xf = np.ascontiguousarray(
        np.asarray(x).astype(f32).reshape(1, N)
    ).astype(BF16_NP)

    gfull = np.asarray(graph, f32)
    in_maps = []
    for i in range(n_cores):
        sl = slice(i * R, (i + 1) * R)
        in_maps.append(
            {
                "g": np.ascontiguousarray(gfull[sl]),
                "xf": xf,
                "mneg": mneg,
                "t2": t2,
                "t6": t6,
                "t7": t7,
                "t5c": t5c,
            }
        )
    return in_maps


def run(inputs: dict, trace: bool = False):
    """Run the distributed kernel on hardware; returns (output, BassKernelResults)."""
    graph = np.asarray(inputs["graph"])
    N = graph.shape[0]
    D = inputs["theta1"].shape[1]
    T = int(inputs["T"])
    v = int(inputs["v"])
    R = N // N_CORES

    nc = get_program(R, N, D, T, v, N_CORES)
    in_maps = make_in_maps(
        graph,
        inputs["x"],
        inputs["theta1"],
        inputs["theta2"],
        inputs["theta3"],
        inputs["theta4"],
        inputs["theta5"],
        inputs["theta6"],
        inputs["theta7"],
        N_CORES,
    )
    res = run_bass_kernel_spmd(
        nc, in_maps, core_ids=list(range(N_CORES)), trace=trace
    )
    out = np.asarray(res.results[0]["out"], np.float32).reshape(1)
    return out, res


def kernel(**inputs) -> np.ndarray:
    out, _ = run(inputs, trace=False)
    return out


# revision 37
# speedup vs baseline: 1.0408x; 1.0408x over previous
"""Distributed Trainium2 (Bass/Tile) kernel for the DQN-style GNN message-passing
module.

Full-input contract: ``kernel(**inputs)`` takes the unsharded inputs exactly as
produced by ``setup_inputs()`` and returns the full output (shape ``(1,)``).

Strategy (v3 — replicated iterations, single tiny AllGather):
  - graph [N, N] row-sharded across 8 cores -> [R, N] per core (R = N/8).
  - Each core streams its shard once and computes per-row chunked sums
    s_abs_k[r] (vector tensor_reduce w/ absolute value) and s_sum_k[r]
    (scalar Copy-activation accumulate), transposes them per 128-row block
    (tensor engine), then combines the chunks with a tiny ones-matmul into
    [s_abs; s_sum] = [2, R] and casts to bf16.
  - ONE AllGather of 4KB/rank shares all row sums; every core then runs ALL
    T iterations + the readout REPLICATED with zero further collectives (the
    v1 baseline instead serialized 4 AllReduces at 10-40us each after
    streaming).

Math (the reference's exact relu identity):
  c[r, :] = s_abs[r] * A + s_sum[r] * B,  A = 0.5*|theta4| @ theta3,
                                          B = 0.5* theta4  @ theta3
  a[r, :] = xf[r] * theta1

  One state tile US [98, N] (bf16) stacks u^T (rows 0:64) over xf (row 64),
  zero padding (rows 65:96 — engine APs need 32-aligned partition starts),
  and [s_abs; s_sum] (rows 96:98).  With the host-built stationary
  M = [-theta2; theta1; 0...; A; B], one matmul computes
    pre^T = M^T @ US = (a + c - u @ theta2)^T
  and u' = relu(pre + z), z = (S_t @ theta2)^T, where S_t = sum_n u_t[n] is
  accumulated locally by the relu pass.  Matmul time is free-dim-bound, so
  the extra contraction rows are free.  u is updated in place (the Tile
  framework tracks subtile WAR/RAW deps per 512-col chunk).
"""

from contextlib import ExitStack

import ml_dtypes
import numpy as np

import concourse.bass as bass
import concourse.tile as tile
from concourse import bacc, mybir
from concourse.bass_utils import run_bass_kernel_spmd

F32 = mybir.dt.float32
BF16 = mybir.dt.bfloat16
AX = mybir.AxisListType
ALU = mybir.AluOpType
ACTF = mybir.ActivationFunctionType
BF16_NP = ml_dtypes.bfloat16

N_CORES = 8
DIM = 64
NCH = 8  # column chunks per row during streaming (chunk = C // NCH columns)
DEBUG_DUMP = False  # extra outputs for numerical debugging

_program_cache: dict = {}


def build_program(R: int, C: int, D: int, T: int, v: int, n_cores: int = N_CORES):
    """Build + compile the per-core SPMD Bass program.

    R: local rows (N / n_cores), C: row length (= N), D: dim, T: iterations,
    v: readout node index (baked in; it is a host-side scalar input).
    """
    assert R % 128 == 0 and D == 64 and C % NCH == 0
    NB = R // 128  # 128-row blocks per core
    CH = C // NCH  # columns per chunk
    # State-tile rows: [u (0:64); xf (64); zero pad (65:96); s_abs; s_sum].
    SOFF = 96
    KM = SOFF + 2  # 98
    FB = 512  # matmul chunk (one PSUM bank of f32)
    RB = 1024  # relu block (2 matmul chunks per activation instruction)
    NBLK = C // RB

    nc = bacc.Bacc(
        "TRN2",
        target_bir_lowering=False,
        debug=False,
        enable_asserts=False,
        num_devices=n_cores,
    )
    g_d = nc.dram_tensor("g", [R, C], F32, kind="ExternalInput")
    xf_d = nc.dram_tensor("xf", [1, C], BF16, kind="ExternalInput")
    mneg_d = nc.dram_tensor("mneg", [KM, D], BF16, kind="ExternalInput")
    t2_d = nc.dram_tensor("t2", [D, D], F32, kind="ExternalInput")
    t6_d = nc.dram_tensor("t6", [D, D], F32, kind="ExternalInput")
    t7_d = nc.dram_tensor("t7", [D, D], BF16, kind="ExternalInput")
    t5c_d = nc.dram_tensor("t5c", [2 * D, 1], F32, kind="ExternalInput")
    out_d = nc.dram_tensor("out", [1, 1], F32, kind="ExternalOutput")
    if DEBUG_DUMP:
        sag_d = nc.dram_tensor("sag_dump", [2 * NCH, R], BF16, kind="ExternalOutput")
        sdump_d = nc.dram_tensor("s_dump", [D, T], F32, kind="ExternalOutput")
    ident_d = nc.inline_tensor(np.eye(128, dtype=np.float32), name="ident")
    cmb_np = np.zeros((2 * NCH, 2), np.float32)
    cmb_np[:NCH, 0] = 1.0
    cmb_np[NCH:, 1] = 1.0
    cmb_d = nc.inline_tensor(cmb_np.astype(BF16_NP), name="cmb")

    rg = [list(range(n_cores))]

    with ExitStack() as ctx:
        tc = ctx.enter_context(tile.TileContext(nc))
        const = ctx.enter_context(tc.tile_pool(name="const", bufs=1))
        small = ctx.enter_context(tc.tile_pool(name="small", bufs=1))
        gp = ctx.enter_context(tc.tile_pool(name="gp", bufs=5))
        stp = ctx.enter_context(tc.tile_pool(name="stp", bufs=2))
        spp = ctx.enter_context(tc.tile_pool(name="spp", bufs=2))
        slp = ctx.enter_context(tc.tile_pool(name="sl", bufs=2))
        dram = ctx.enter_context(tc.tile_pool(name="dram", bufs=1, space="DRAM"))

        # ---- warm-up collectives (absorb ncfw cold-start under streaming;
        # the mesh collective reaches steady-state latency after ~2 ops)
        dwi = dram.tile([2, 128], BF16, tag="dwi")
        dwo = dram.tile([2 * n_cores, 128], BF16, tag="dwo")
        for _ in range(2):
            nc.gpsimd.collective_compute(
                "AllGather",
                ALU.bypass,
                replica_groups=rg,
                ins=[dwi[:].opt()],
                outs=[dwo[:].opt()],
            )

        # ---- constants / persistent tiles
        ident = const.tile([128, 128], F32)
        nc.scalar.dma_start(out=ident[:], in_=ident_d.ap())
        cmb = const.tile([2 * NCH, 2], BF16)
        nc.scalar.dma_start(out=cmb[:], in_=cmb_d.ap())
        Mb = const.tile([KM, D], BF16)
        nc.scalar.dma_start(out=Mb[:], in_=mneg_d.ap())
        t2 = const.tile([D, D], F32)
        nc.scalar.dma_start(out=t2[:], in_=t2_d.ap())
        t6 = const.tile([D, D], F32)
        nc.scalar.dma_start(out=t6[:], in_=t6_d.ap())
        t7b = const.tile([D, D], BF16)
        nc.scalar.dma_start(out=t7b[:], in_=t7_d.ap())
        t5c = const.tile([2 * D, 1], F32)
        nc.scalar.dma_start(out=t5c[:], in_=t5c_d.ap())
        z0 = const.tile([D, 1], F32)
        nc.vector.memset(z0[:], 0.0)
        zfb = const.tile([D, RB], F32)
        nc.vector.memset(zfb[:], 0.0)

        # single in-place state tile
        US = small.tile([KM, C], BF16)
        nc.vector.memset(US[0:D, :], 0.0)
        nc.vector.memset(US[D:SOFF, :], 0.0)
        nc.scalar.dma_start(out=US[D : D + 1, :], in_=xf_d.ap())

        # per-core chunked row sums, transposed: rows = [abs_0..; sum_0..]
        SAg = small.tile([2 * NCH, R], BF16)
        cinS = small.tile([2, R], BF16)

        # ---- phase 1: stream graph, reduce rows, transpose per block
        with tc.tile_pool(name="psT", bufs=2, space="PSUM") as psT, tc.tile_pool(
            name="psC", bufs=2, space="PSUM"
        ) as psC:
            for b in range(NB):
                gt = gp.tile([128, C], F32, tag="gt")
                nsub = 8 if b == NB - 1 else 2
                sw = C // nsub
                for h in range(nsub):
                    nc.sync.dma_start(
                        out=gt[:, h * sw : (h + 1) * sw],
                        in_=g_d.ap()[b * 128 : (b + 1) * 128, h * sw : (h + 1) * sw],
                    )
                SPA = spp.tile([128, 2 * NCH], F32, tag="spa")
                for k in range(NCH):
                    nc.vector.tensor_reduce(
                        out=SPA[:, k : k + 1],
                        in_=gt[:, k * CH : (k + 1) * CH],
                        axis=AX.X,
                        op=ALU.add,
                        apply_absolute_value=True,
                    )
                    st = stp.tile([128, CH], BF16, tag="st")
                    nc.scalar.activation(
                        out=st[:],
                        in_=gt[:, k * CH : (k + 1) * CH],
                        func=ACTF.Copy,
                        accum_out=SPA[:, NCH + k : NCH + k + 1],
                    )
                tb = psT.tile([2 * NCH, 128], F32, tag="tb")
                nc.tensor.transpose(out=tb[:], in_=SPA[:], identity=ident[:])
                nc.vector.tensor_copy(SAg[:, b * 128 : (b + 1) * 128], tb[:])
                if b == NB // 2 - 1 or b == NB - 1:
                    # combine chunk sums -> [s_abs; s_sum] bf16 as soon as a
                    # half of SAg is ready (avoids tensor-queue HOL blocking)
                    h = 0 if b == NB // 2 - 1 else FB
                    cp = psC.tile([2, FB], F32, tag="cp")
                    nc.tensor.matmul(
                        cp[:], lhsT=cmb[:], rhs=SAg[:, h : h + FB],
                        start=True, stop=True,
                    )
                    nc.vector.tensor_copy(cinS[:, h : h + FB], cp[:])
            if DEBUG_DUMP:
                nc.scalar.dma_start(out=sag_d.ap(), in_=SAg[:])

        # ---- share row sums: one tiny AllGather
        cin = dram.tile([2, R], BF16, tag="cin")
        cout = dram.tile([2 * n_cores, R], BF16, tag="cout")
        nc.sync.dma_start(out=cin[:], in_=cinS[:])
        nc.gpsimd.collective_compute(
            "AllGather",
            ALU.bypass,
            replica_groups=rg,
            ins=[cin[:].opt()],
            outs=[cout[:].opt()],
        )
        for i in range(n_cores):
            nc.sync.dma_start(
                out=US[SOFF:KM, i * R : (i + 1) * R],
                in_=cout[2 * i : 2 * i + 2, :],
            )
        # keep the PE HAM-warm through the AllGather window so the iteration
        # matmuls run at 2.4 GHz (reads cinS, so this schedules after phase 1)
        with tc.tile_pool(name="psW", bufs=1, space="PSUM") as psW:
            jp = psW.tile([D, FB], F32, tag="jp")
            for _ in range(20):
                nc.tensor.matmul(
                    jp[:], lhsT=cinS[0:2, 0:D], rhs=cinS[:, 0:FB],
                    start=True, stop=True,
                )

        # ---- T replicated iterations over all C nodes (u updated in place)
        psI_ctx = tc.tile_pool(name="psI", bufs=3, space="PSUM")
        psZ_ctx = tc.tile_pool(name="psZ", bufs=1, space="PSUM")
        psI = psI_ctx.__enter__()
        psZ = psZ_ctx.__enter__()
        zs = z0
        S_last = None
        if DEBUG_DUMP:
            sdt = small.tile([D, T], F32)
        for t in range(T):
            SL = slp.tile([D, NBLK], F32, tag="SL", name=f"SL{t}")
            for blk in range(NBLK):
                cols = slice(blk * RB, (blk + 1) * RB)
                ps = psI.tile([D, RB], F32, tag="ps")
                for h in range(2):
                    nc.tensor.matmul(
                        ps[:, h * FB : (h + 1) * FB],
                        lhsT=Mb[:],
                        rhs=US[:, blk * RB + h * FB : blk * RB + (h + 1) * FB],
                        start=True,
                        stop=True,
                    )
                if blk % 2 == 0:
                    nc.scalar.activation(
                        out=US[0:D, cols],
                        in_=ps[:],
                        func=ACTF.Relu,
                        bias=zs[:, 0:1],
                        accum_out=SL[:, blk : blk + 1],
                    )
                else:
                    # relu(ps + z) on DVE: (ps add z) max zeros, accum -> SL
                    nc.vector.scalar_tensor_tensor(
                        out=US[0:D, cols],
                        in0=ps[:],
                        scalar=zs[:, 0:1],
                        in1=zfb[:],
                        op0=ALU.add,
                        op1=ALU.max,
                        accum_out=SL[:, blk : blk + 1],
                    )
            S = slp.tile([D, 1], F32, tag="S", name=f"S{t}")
            nc.vector.tensor_reduce(out=S[:], in_=SL[:], axis=AX.X, op=ALU.add)
            S_last = S
            if DEBUG_DUMP:
                nc.vector.tensor_copy(sdt[:, t : t + 1], S[:])
            if t < T - 1:
                zp = psZ.tile([D, 1], F32, tag="zp")
                nc.tensor.matmul(zp[:], lhsT=t2[:], rhs=S[:], start=True, stop=True)
                znew = slp.tile([D, 1], F32, tag="zs", name=f"zs{t}")
                nc.scalar.copy(znew[:], zp[:])
                zs = znew
        psZ_ctx.__exit__(None, None, None)
        psI_ctx.__exit__(None, None, None)
        if DEBUG_DUMP:
            nc.scalar.dma_start(out=sdump_d.ap(), in_=sdt[:])

        # ---- final readout (fully local; u4 = US rows 0:64, S4 = S_last)
        with tc.tile_pool(name="psF", bufs=1, space="PSUM") as psF:
            q = psF.tile([2 * D, 1], F32, tag="q")
            nc.tensor.matmul(
                q[0:D, :], lhsT=t6[:], rhs=S_last[:], start=True, stop=True
            )
            nc.tensor.matmul(
                q[D : 2 * D, :],
                lhsT=t7b[:],
                rhs=US[0:D, v : v + 1],
                start=True,
                stop=True,
            )
            rq = small.tile([2 * D, 1], F32)
            nc.scalar.activation(out=rq[:], in_=q[:], func=ACTF.Relu)
            res = psF.tile([1, 1], F32, tag="res")
            nc.tensor.matmul(res[:], lhsT=rq[:], rhs=t5c[:], start=True, stop=True)
            ress = small.tile([1, 1], F32)
            nc.scalar.copy(ress[:], res[:])
            nc.scalar.dma_start(out=out_d.ap(), in_=ress[:])

    nc.compile()
    return nc


def get_program(R: int, C: int, D: int, T: int, v: int, n_cores: int = N_CORES):
    key = (R, C, D, T, v, n_cores)
    if key not in _program_cache:
        _program_cache[key] = build_program(R, C, D, T, v, n_cores)
    return _program_cache[key]


def make_in_maps(graph, x, theta1, theta2, theta3, theta4, theta5, theta6, theta7,
                 n_cores: int = N_CORES):
    """Host-side sharding + tiny theta preprocessing."""
    N = graph.shape[0]
    D = theta1.shape[1]
    R = N // n_cores
    f32 = np.float32

    t4 = np.asarray(theta4, f32)[0]
    t3 = np.asarray(theta3, f32)
    A = 0.5 * (np.abs(t4) @ t3)
    B = 0.5 * (t4 @ t3)
    t2 = np.ascontiguousarray(np.asarray(theta2, f32))
    mneg = np.ascontiguousarray(
        np.concatenate(
            [-t2, np.asarray(theta1, f32), np.zeros((31, D), f32),
             A[None, :], B[None, :]],
            axis=0,
        )
    ).astype(BF16_NP)  # (98, D)
    t5c = np.ascontiguousarray(np.asarray(theta5, f32).reshape(2 * D, 1))
    t6 = np.ascontiguousarray(np.asarray(theta6, f32))
    t7 = np.ascontiguousarray(np.asarray(theta7, f32)).astype(BF16_NP)
    xf = np.ascontiguousarray(
        np.asarray(x).astype(f32).reshape(1, N)
    ).astype(BF16_NP)

    gfull = np.asarray(graph, f32)
    in_maps = []
    for i in range(n_cores):
        sl = slice(i * R, (i + 1) * R)
        in_maps.append(
            {
                "g": np.ascontiguousarray(gfull[sl]),
                "xf": xf,
                "mneg": mneg,
                "t2": t2,
                "t6": t6,
                "t7": t7,
                "t5c": t5c,
            }
        )
    return in_maps


def run(inputs: dict, trace: bool = False):
    """Run the distributed kernel on hardware; returns (output, BassKernelResults)."""
    graph = np.asarray(inputs["graph"])
    N = graph.shape[0]
    D = inputs["theta1"].shape[1]
    T = int(inputs["T"])
    v = int(inputs["v"])
    R = N // N_CORES

    nc = get_program(R, N, D, T, v, N_CORES)
    in_maps = make_in_maps(
        graph,
        inputs["x"],
        inputs["theta1"],
        inputs["theta2"],
        inputs["theta3"],
        inputs["theta4"],
        inputs["theta5"],
        inputs["theta6"],
        inputs["theta7"],
        N_CORES,
    )
    res = run_bass_kernel_spmd(
        nc, in_maps, core_ids=list(range(N_CORES)), trace=trace
    )
    out = np.asarray(res.results[0]["out"], np.float32).reshape(1)
    return out, res


def kernel(**inputs) -> np.ndarray:
    out, _ = run(inputs, trace=False)
    return out


# revision 38
# speedup vs baseline: 1.0821x; 1.0397x over previous
"""Distributed Trainium2 (Bass/Tile) kernel for the DQN-style GNN message-passing
module.

Full-input contract: ``kernel(**inputs)`` takes the unsharded inputs exactly as
produced by ``setup_inputs()`` and returns the full output (shape ``(1,)``).

Strategy (v3 — replicated iterations, single tiny AllGather):
  - graph [N, N] row-sharded across 8 cores -> [R, N] per core (R = N/8).
  - Each core streams its shard once and computes per-row chunked sums
    s_abs_k[r] (vector tensor_reduce w/ absolute value) and s_sum_k[r]
    (scalar Copy-activation accumulate), transposes them per 128-row block
    (tensor engine), then combines the chunks with a tiny ones-matmul into
    [s_abs; s_sum] = [2, R] and casts to bf16.
  - ONE AllGather of 4KB/rank shares all row sums; every core then runs ALL
    T iterations + the readout REPLICATED with zero further collectives (the
    v1 baseline instead serialized 4 AllReduces at 10-40us each after
    streaming).

Math (the reference's exact relu identity):
  c[r, :] = s_abs[r] * A + s_sum[r] * B,  A = 0.5*|theta4| @ theta3,
                                          B = 0.5* theta4  @ theta3
  a[r, :] = xf[r] * theta1

  One state tile US [98, N] (bf16) stacks u^T (rows 0:64) over xf (row 64),
  zero padding (rows 65:96 — engine APs need 32-aligned partition starts),
  and [s_abs; s_sum] (rows 96:98).  With the host-built stationary
  M = [-theta2; theta1; 0...; A; B], one matmul computes
    pre^T = M^T @ US = (a + c - u @ theta2)^T
  and u' = relu(pre + z), z = (S_t @ theta2)^T, where S_t = sum_n u_t[n] is
  accumulated locally by the relu pass.  Matmul time is free-dim-bound, so
  the extra contraction rows are free.  u is updated in place (the Tile
  framework tracks subtile WAR/RAW deps per 512-col chunk).
"""

from contextlib import ExitStack

import ml_dtypes
import numpy as np

import concourse.bass as bass
import concourse.tile as tile
from concourse import bacc, mybir
from concourse.bass_utils import run_bass_kernel_spmd

F32 = mybir.dt.float32
BF16 = mybir.dt.bfloat16
AX = mybir.AxisListType
ALU = mybir.AluOpType
ACTF = mybir.ActivationFunctionType
BF16_NP = ml_dtypes.bfloat16

N_CORES = 8
DIM = 64
NCH = 8  # column chunks per row during streaming (chunk = C // NCH columns)
DEBUG_DUMP = False  # extra outputs for numerical debugging

_program_cache: dict = {}


def build_program(R: int, C: int, D: int, T: int, v: int, n_cores: int = N_CORES):
    """Build + compile the per-core SPMD Bass program.

    R: local rows (N / n_cores), C: row length (= N), D: dim, T: iterations,
    v: readout node index (baked in; it is a host-side scalar input).
    """
    assert R % 128 == 0 and D == 64 and C % NCH == 0
    NB = R // 128  # 128-row blocks per core
    CH = C // NCH  # columns per chunk
    # State-tile rows: [u (0:64); xf (64); zero pad (65:96); s_abs; s_sum].
    SOFF = 96
    KM = SOFF + 2  # 98
    FB = 512  # matmul chunk (one PSUM bank of f32)
    RB = 1024  # relu block (2 matmul chunks per activation instruction)
    NBLK = C // RB

    nc = bacc.Bacc(
        "TRN2",
        target_bir_lowering=False,
        debug=False,
        enable_asserts=False,
        num_devices=n_cores,
    )
    g_d = nc.dram_tensor("g", [R, C], F32, kind="ExternalInput")
    xf_d = nc.dram_tensor("xf", [1, C], BF16, kind="ExternalInput")
    mneg_d = nc.dram_tensor("mneg", [KM, D], BF16, kind="ExternalInput")
    t2_d = nc.dram_tensor("t2", [D, D], F32, kind="ExternalInput")
    t6_d = nc.dram_tensor("t6", [D, D], F32, kind="ExternalInput")
    t7_d = nc.dram_tensor("t7", [D, D], BF16, kind="ExternalInput")
    t5c_d = nc.dram_tensor("t5c", [2 * D, 1], F32, kind="ExternalInput")
    out_d = nc.dram_tensor("out", [1, 1], F32, kind="ExternalOutput")
    if DEBUG_DUMP:
        sag_d = nc.dram_tensor("sag_dump", [2 * NCH, R], BF16, kind="ExternalOutput")
        sdump_d = nc.dram_tensor("s_dump", [D, T], F32, kind="ExternalOutput")
    ident_d = nc.inline_tensor(np.eye(128, dtype=np.float32), name="ident")
    cmb_np = np.zeros((2 * NCH, 2), np.float32)
    cmb_np[:NCH, 0] = 1.0
    cmb_np[NCH:, 1] = 1.0
    cmb_d = nc.inline_tensor(cmb_np.astype(BF16_NP), name="cmb")

    rg = [list(range(n_cores))]

    with ExitStack() as ctx:
        tc = ctx.enter_context(tile.TileContext(nc))
        const = ctx.enter_context(tc.tile_pool(name="const", bufs=1))
        small = ctx.enter_context(tc.tile_pool(name="small", bufs=1))
        gp = ctx.enter_context(tc.tile_pool(name="gp", bufs=5))
        stp = ctx.enter_context(tc.tile_pool(name="stp", bufs=2))
        spp = ctx.enter_context(tc.tile_pool(name="spp", bufs=2))
        slp = ctx.enter_context(tc.tile_pool(name="sl", bufs=2))
        dram = ctx.enter_context(tc.tile_pool(name="dram", bufs=1, space="DRAM"))

        # ---- warm-up collectives (absorb ncfw cold-start under streaming;
        # the mesh collective reaches steady-state latency after ~2 ops)
        dwi = dram.tile([2, 128], BF16, tag="dwi")
        dwo = dram.tile([2 * n_cores, 128], BF16, tag="dwo")
        for _ in range(1):
            nc.gpsimd.collective_compute(
                "AllGather",
                ALU.bypass,
                replica_groups=rg,
                ins=[dwi[:].opt()],
                outs=[dwo[:].opt()],
            )

        # ---- constants / persistent tiles
        ident = const.tile([128, 128], F32)
        nc.scalar.dma_start(out=ident[:], in_=ident_d.ap())
        cmb = const.tile([2 * NCH, 2], BF16)
        nc.scalar.dma_start(out=cmb[:], in_=cmb_d.ap())
        Mb = const.tile([KM, D], BF16)
        nc.scalar.dma_start(out=Mb[:], in_=mneg_d.ap())
        t2 = const.tile([D, D], F32)
        nc.scalar.dma_start(out=t2[:], in_=t2_d.ap())
        t6 = const.tile([D, D], F32)
        nc.scalar.dma_start(out=t6[:], in_=t6_d.ap())
        t7b = const.tile([D, D], BF16)
        nc.scalar.dma_start(out=t7b[:], in_=t7_d.ap())
        t5c = const.tile([2 * D, 1], F32)
        nc.scalar.dma_start(out=t5c[:], in_=t5c_d.ap())
        z0 = const.tile([D, 1], F32)
        nc.vector.memset(z0[:], 0.0)
        zfb = const.tile([D, RB], F32)
        nc.vector.memset(zfb[:], 0.0)

        # single in-place state tile
        US = small.tile([KM, C], BF16)
        nc.vector.memset(US[0:D, :], 0.0)
        nc.vector.memset(US[D:SOFF, :], 0.0)
        nc.scalar.dma_start(out=US[D : D + 1, :], in_=xf_d.ap())

        # per-core chunked row sums, transposed: rows = [abs_0..; sum_0..]
        SAg = small.tile([2 * NCH, R], BF16)
        cinS = small.tile([2, R], BF16)

        # ---- phase 1: stream graph, reduce rows, transpose per block
        with tc.tile_pool(name="psT", bufs=2, space="PSUM") as psT, tc.tile_pool(
            name="psC", bufs=2, space="PSUM"
        ) as psC:
            for b in range(NB):
                gt = gp.tile([128, C], F32, tag="gt")
                nsub = 8 if b == NB - 1 else 2
                sw = C // nsub
                for h in range(nsub):
                    nc.sync.dma_start(
                        out=gt[:, h * sw : (h + 1) * sw],
                        in_=g_d.ap()[b * 128 : (b + 1) * 128, h * sw : (h + 1) * sw],
                    )
                SPA = spp.tile([128, 2 * NCH], F32, tag="spa")
                for k in range(NCH):
                    nc.vector.tensor_reduce(
                        out=SPA[:, k : k + 1],
                        in_=gt[:, k * CH : (k + 1) * CH],
                        axis=AX.X,
                        op=ALU.add,
                        apply_absolute_value=True,
                    )
                    st = stp.tile([128, CH], BF16, tag="st")
                    nc.scalar.activation(
                        out=st[:],
                        in_=gt[:, k * CH : (k + 1) * CH],
                        func=ACTF.Copy,
                        accum_out=SPA[:, NCH + k : NCH + k + 1],
                    )
                tb = psT.tile([2 * NCH, 128], F32, tag="tb")
                nc.tensor.transpose(out=tb[:], in_=SPA[:], identity=ident[:])
                nc.vector.tensor_copy(SAg[:, b * 128 : (b + 1) * 128], tb[:])
                if b == NB // 2 - 1 or b == NB - 1:
                    # combine chunk sums -> [s_abs; s_sum] bf16 as soon as a
                    # half of SAg is ready (avoids tensor-queue HOL blocking)
                    h = 0 if b == NB // 2 - 1 else FB
                    cp = psC.tile([2, FB], F32, tag="cp")
                    nc.tensor.matmul(
                        cp[:], lhsT=cmb[:], rhs=SAg[:, h : h + FB],
                        start=True, stop=True,
                    )
                    nc.vector.tensor_copy(cinS[:, h : h + FB], cp[:])
            if DEBUG_DUMP:
                nc.scalar.dma_start(out=sag_d.ap(), in_=SAg[:])

        # ---- share row sums: one tiny AllGather
        cin = dram.tile([2, R], BF16, tag="cin")
        cout = dram.tile([2 * n_cores, R], BF16, tag="cout")
        nc.sync.dma_start(out=cin[:], in_=cinS[:])
        nc.gpsimd.collective_compute(
            "AllGather",
            ALU.bypass,
            replica_groups=rg,
            ins=[cin[:].opt()],
            outs=[cout[:].opt()],
        )
        for i in range(n_cores):
            nc.sync.dma_start(
                out=US[SOFF:KM, i * R : (i + 1) * R],
                in_=cout[2 * i : 2 * i + 2, :],
            )
        # keep the PE HAM-warm through the AllGather window so the iteration
        # matmuls run at 2.4 GHz (reads cinS, so this schedules after phase 1)
        with tc.tile_pool(name="psW", bufs=1, space="PSUM") as psW:
            jp = psW.tile([D, FB], F32, tag="jp")
            for _ in range(20):
                nc.tensor.matmul(
                    jp[:], lhsT=cinS[0:2, 0:D], rhs=cinS[:, 0:FB],
                    start=True, stop=True,
                )

        # ---- T replicated iterations over all C nodes (u updated in place)
        psI_ctx = tc.tile_pool(name="psI", bufs=3, space="PSUM")
        psZ_ctx = tc.tile_pool(name="psZ", bufs=1, space="PSUM")
        psI = psI_ctx.__enter__()
        psZ = psZ_ctx.__enter__()
        zs = z0
        S_last = None
        if DEBUG_DUMP:
            sdt = small.tile([D, T], F32)
        for t in range(T):
            SL = slp.tile([D, NBLK], F32, tag="SL", name=f"SL{t}")
            for blk in range(NBLK):
                cols = slice(blk * RB, (blk + 1) * RB)
                ps = psI.tile([D, RB], F32, tag="ps")
                for h in range(2):
                    nc.tensor.matmul(
                        ps[:, h * FB : (h + 1) * FB],
                        lhsT=Mb[:],
                        rhs=US[:, blk * RB + h * FB : blk * RB + (h + 1) * FB],
                        start=True,
                        stop=True,
                    )
                if blk % 2 == 0:
                    nc.scalar.activation(
                        out=US[0:D, cols],
                        in_=ps[:],
                        func=ACTF.Relu,
                        bias=zs[:, 0:1],
                        accum_out=SL[:, blk : blk + 1],
                    )
                else:
                    # relu(ps + z) on DVE: (ps add z) max zeros, accum -> SL
                    nc.vector.scalar_tensor_tensor(
                        out=US[0:D, cols],
                        in0=ps[:],
                        scalar=zs[:, 0:1],
                        in1=zfb[:],
                        op0=ALU.add,
                        op1=ALU.max,
                        accum_out=SL[:, blk : blk + 1],
                    )
            S = slp.tile([D, 1], F32, tag="S", name=f"S{t}")
            nc.vector.tensor_reduce(out=S[:], in_=SL[:], axis=AX.X, op=ALU.add)
            S_last = S
            if DEBUG_DUMP:
                nc.vector.tensor_copy(sdt[:, t : t + 1], S[:])
            if t < T - 1:
                zp = psZ.tile([D, 1], F32, tag="zp")
                nc.tensor.matmul(zp[:], lhsT=t2[:], rhs=S[:], start=True, stop=True)
                znew = slp.tile([D, 1], F32, tag="zs", name=f"zs{t}")
                nc.scalar.copy(znew[:], zp[:])
                zs = znew
        psZ_ctx.__exit__(None, None, None)
        psI_ctx.__exit__(None, None, None)
        if DEBUG_DUMP:
            nc.scalar.dma_start(out=sdump_d.ap(), in_=sdt[:])

        # ---- final readout (fully local; u4 = US rows 0:64, S4 = S_last)
        with tc.tile_pool(name="psF", bufs=1, space="PSUM") as psF:
            q = psF.tile([2 * D, 1], F32, tag="q")
            nc.tensor.matmul(
                q[0:D, :], lhsT=t6[:], rhs=S_last[:], start=True, stop=True
            )
            nc.tensor.matmul(
                q[D : 2 * D, :],
                lhsT=t7b[:],
                rhs=US[0:D, v : v + 1],
                start=True,
                stop=True,
            )
            rq = small.tile([2 * D, 1], F32)
            nc.scalar.activation(out=rq[:], in_=q[:], func=ACTF.Relu)
            res = psF.tile([1, 1], F32, tag="res")
            nc.tensor.matmul(res[:], lhsT=rq[:], rhs=t5c[:], start=True, stop=True)
            ress = small.tile([1, 1], F32)
            nc.scalar.copy(ress[:], res[:])
            nc.scalar.dma_start(out=out_d.ap(), in_=ress[:])

    nc.compile()
    return nc


def get_program(R: int, C: int, D: int, T: int, v: int, n_cores: int = N_CORES):
    key = (R, C, D, T, v, n_cores)
    if key not in _program_cache:
        _program_cache[key] = build_program(R, C, D, T, v, n_cores)
    return _program_cache[key]


def make_in_maps(graph, x, theta1, theta2, theta3, theta4, theta5, theta6, theta7,
                 n_cores: int = N_CORES):
    """Host-side sharding + tiny theta preprocessing."""
    N = graph.shape[0]
    D = theta1.shape[1]
    R = N // n_cores
    f32 = np.float32

    t4 = np.asarray(theta4, f32)[0]
    t3 = np.asarray(theta3, f32)
    A = 0.5 * (np.abs(t4) @ t3)
    B = 0.5 * (t4 @ t3)
    t2 = np.ascontiguousarray(np.asarray(theta2, f32))
    mneg = np.ascontiguousarray(
        np.concatenate(
            [-t2, np.asarray(theta1, f32), np.zeros((31, D), f32),
             A[None, :], B[None, :]],
            axis=0,
        )
    ).astype(BF16_NP)  # (98, D)
    t5c = np.ascontiguousarray(np.asarray(theta5, f32).reshape(2 * D, 1))
    t6 = np.ascontiguousarray(np.asarray(theta6, f32))
    t7 = np.ascontiguousarray(np.asarray(theta7, f32)).astype(BF16_NP)
    xf = np.ascontiguousarray(
        np.asarray(x).astype(f32).reshape(1, N)
    ).astype(BF16_NP)

    gfull = np.asarray(graph, f32)
    in_maps = []
    for i in range(n_cores):
        sl = slice(i * R, (i + 1) * R)
        in_maps.append(
            {
                "g": np.ascontiguousarray(gfull[sl]),
                "xf": xf,
                "mneg": mneg,
                "t2": t2,
                "t6": t6,
                "t7": t7,
                "t5c": t5c,
            }
        )
    return in_maps


def run(inputs: dict, trace: bool = False):
    """Run the distributed kernel on hardware; returns (output, BassKernelResults)."""
    graph = np.asarray(inputs["graph"])
    N = graph.shape[0]
    D = inputs["theta1"].shape[1]
    T = int(inputs["T"])
    v = int(inputs["v"])
    R = N // N_CORES

    nc = get_program(R, N, D, T, v, N_CORES)
    in_maps = make_in_maps(
        graph,
        inputs["x"],
        inputs["theta1"],
        inputs["theta2"],
        inputs["theta3"],
        inputs["theta4"],
        inputs["theta5"],
        inputs["theta6"],
        inputs["theta7"],
        N_CORES,
    )
    res = run_bass_kernel_spmd(
        nc, in_maps, core_ids=list(range(N_CORES)), trace=trace
    )
    out = np.asarray(res.results[0]["out"], np.float32).reshape(1)
    return out, res


def kernel(**inputs) -> np.ndarray:
    out, _ = run(inputs, trace=False)
    return out
